# revision 35
# baseline (speedup 1.0000x reference)
"""Causal self-attention TRN2 kernel (bf16, software-pipelined).

Full module: x[4,2048,1024] @ W_qkv[1024,3072] -> heads(16, d=64) causal attn
-> @ W_proj[1024,1024].

Sharding: 8 cores = 4 batches x 2 head-groups (8 heads each), tensor-parallel
over heads. Each core computes q/k/v for its 8 heads, causal attention, and a
partial projection (row-sharded W_proj). The two partials per batch are summed
on the host (no on-device collectives).

Design (v1 measured 540us/pass on HW; this version ~370us; TimelineSim 279us
with PE busy 236us ~= the causal-attention FLOP roofline at 78.6 TF/s bf16):
  - All matmul operands bf16: PE rate equals f32r, no low-pstate penalty,
    half the SBUF/DMA footprint. End-to-end bf16 numpy model: 0.41% max err;
    measured 0.38% on HW vs the 2% gate (output is returned as bf16 and
    upcast on the host, +0.2%).
  - Diagonal skip: for a diagonal k-tile at block offset d, the first 128*d
    q-columns are fully masked -> S matmul, exp and AV all run on the partial
    width only. No zero-fill DMAs (v1 fetched zeros from HBM on the exp->AV
    critical path).
  - Attention processed per (head-pair, k-tile) step: two 64-contraction S
    matmuls write one [128,1024] PSUM tile (head A | head B), ONE exp covers
    both heads, one strided DVE multiply applies the causal mask to both
    diagonal blocks, two AV matmuls accumulate per-head [65,512] y tiles
    (V|ones trick: row 64 = softmax denominator). Fewer, larger ops measured
    faster on HW than per-head [128,512] steps - cross-engine semaphore
    latency dominates over the simulator's estimates.
  - PSUM: 8 banks = scores 2x2 + y-accum 2 + qkv/proj/bcast 2. Dedicated
    pools per phase (v1 shared one pool, serializing QKV behind projection).
  - Software-pipelined emission: per-block windows interleave attention(qc)
    with xq-prefetch/QKV(qc+1) and projection(qc-2..) steps; blocks 2 and 3
    are merged at hp-chain granularity and all projection work is pushed into
    the late (exp-paced) windows so PE stays saturated. Weight/x loads are
    one or two big host-packed DMAs each ([128, chunk, cols] layout).
  - PSUM->SBUF staging on DVE (GPSIMD cannot access PSUM; ACT is the
    exp-paced engine), mask multiplies on DVE, v-ones fill on GPSIMD.

build_nc(repeat=R) emits the whole computation R times (rep r+1's first xq
chunk is sourced from rep r's final output rows, serializing reps) for
wall-clock-differenced device timing.
"""

import numpy as np
import ml_dtypes
from contextlib import ExitStack

import concourse.bass as bass
import concourse.tile as tile
from concourse import mybir, bacc
from concourse.bass_utils import run_bass_kernel_spmd

F32 = mybir.dt.float32
F32R = mybir.dt.float32r
BF16 = mybir.dt.bfloat16
EXP = mybir.ActivationFunctionType.Exp

B, T, C, H, D = 4, 2048, 1024, 16, 64
NCORES = 8
GROUPS = 2            # head groups (tensor-parallel dimension)
HPC = H // GROUPS     # heads per core = 8
FPC = HPC * D         # features per core = 512
SCALE = 1.0 / np.sqrt(D)


def build_nc(repeat=1, serialize_reps=False):
    NC = C // 128     # contraction chunks over C = 8
    NT = T // 128     # token tiles (also k-tiles) = 16
    NQ = T // 512     # query chunks = 4
    NF = FPC // 128   # feature tiles = head pairs = 4
    nc = bacc.Bacc("TRN2", debug=False)
    xT_d = nc.dram_tensor("xT", [128, NC, T], BF16, kind="ExternalInput").ap()
    wq_d = nc.dram_tensor("wq", [128, NC, FPC], BF16, kind="ExternalInput").ap()
    wk_d = nc.dram_tensor("wk", [128, NC, FPC], BF16, kind="ExternalInput").ap()
    wv_d = nc.dram_tensor("wv", [128, NC, FPC], BF16, kind="ExternalInput").ap()
    wp_d = nc.dram_tensor("wp", [128, NF, C], BF16, kind="ExternalInput").ap()
    mk_d = nc.dram_tensor("trimask", [128, 128], BF16, kind="ExternalInput").ap()
    on_d = nc.dram_tensor("ones64", [1, 64], F32R, kind="ExternalInput").ap()
    ov_d = nc.dram_tensor("onesv", [128, HPC], BF16, kind="ExternalInput").ap()
    out_d = nc.dram_tensor("out", [T, C], BF16, kind="ExternalOutput").ap()

    with tile.TileContext(nc) as tc, ExitStack() as ctx:
        p_const = ctx.enter_context(tc.tile_pool(name="p_const", bufs=1))
        p_kt = ctx.enter_context(tc.tile_pool(name="p_kt", bufs=NF))
        p_v65 = ctx.enter_context(tc.tile_pool(name="p_v65", bufs=NT))
        p_w = ctx.enter_context(tc.tile_pool(name="p_w", bufs=3))
        p_wp = ctx.enter_context(tc.tile_pool(name="p_wp", bufs=NF))
        p_xq = ctx.enter_context(tc.tile_pool(name="p_xq", bufs=2))
        p_qtq = ctx.enter_context(tc.tile_pool(name="p_qtq", bufs=2 * NF))
        p_ytq = ctx.enter_context(tc.tile_pool(name="p_ytq", bufs=4 * NF))
        p_pt = ctx.enter_context(tc.tile_pool(name="p_pt", bufs=4))
        p_ys = ctx.enter_context(tc.tile_pool(name="p_ys", bufs=4))
        p_rec = ctx.enter_context(tc.tile_pool(name="p_rec", bufs=2))
        p_ybt = ctx.enter_context(tc.tile_pool(name="p_ybt", bufs=2))
        p_osb = ctx.enter_context(tc.tile_pool(name="p_osb", bufs=2))
        # PSUM: 8 banks = scores 2 + y-accum 2 + qkv 2 + proj/bcast 2
        ps_s = ctx.enter_context(tc.tile_pool(name="ps_s", bufs=2, space="PSUM"))
        ps_y = ctx.enter_context(tc.tile_pool(name="ps_y", bufs=2, space="PSUM"))
        ps_q = ctx.enter_context(tc.tile_pool(name="ps_q", bufs=2, space="PSUM"))

        # constants (loaded once, outside the repeat loop, after the first
        # x/wq quarters — they are not needed until attention starts)
        ones_t = p_const.tile([65, 64], F32R, tag="ones")
        trimask2 = p_const.tile([128, 2, 128], BF16, tag="trimask2")
        onesv = p_const.tile([128, HPC], BF16, tag="onesv")
        consts = {"loaded": False}

        def load_consts():
            if consts["loaded"]:
                return
            consts["loaded"] = True
            nc.sync.dma_start(out=ones_t[64:65, :], in_=on_d[:])
            nc.sync.dma_start(out=trimask2[:, 0, :], in_=mk_d[:])
            nc.sync.dma_start(out=trimask2[:, 1, :], in_=mk_d[:])
            nc.sync.dma_start(out=onesv[:], in_=ov_d[:])

        kt_ = [p_kt.tile([128, T], BF16, tag="kt", name=f"kt{i}")
               for i in range(NF)]
        v65 = [p_v65.tile([128, HPC, 65], BF16, tag="v65", name=f"v65_{i}")
               for i in range(NT)]

        serdep = {"on": False}

        def emit_once():
            qtq = {}   # (f, qc) -> [128, 512] bf16 query quarter (transposed)
            ytq = {}   # (hp, qc) -> [128, 512] bf16 attention-out quarter
            xq = {}    # (c, qc) -> [128, 512] bf16 x chunk (transposed)
            wq_sb = wk_sb = wv_sb = wp_sb = None

            def dma_xq_steps(qc):
                def one():
                    t_ = p_xq.tile([128, NC, 512], BF16, tag="xq",
                                   name=f"xq{qc}")
                    if serdep["on"] and qc == 0:
                        # timing mode: source chunk 0 from the previous
                        # repeat's output rows — every q/k/v contraction
                        # chain starts at chunk 0, so reps serialize
                        nc.sync.dma_start(
                            out=t_[:, 0:1, :],
                            in_=out_d[(NT - 1) * 128:NT * 128, 0:512]
                            .rearrange("p (o f) -> p o f", o=1))
                        nc.sync.dma_start(
                            out=t_[:, 1:NC, :],
                            in_=xT_d[:, 1:NC, qc * 512:(qc + 1) * 512])
                    elif qc == 0:
                        for c4 in range(0, NC, 2):
                            nc.sync.dma_start(
                                out=t_[:, c4:c4 + 2, :],
                                in_=xT_d[:, c4:c4 + 2,
                                         qc * 512:(qc + 1) * 512])
                    else:
                        nc.sync.dma_start(
                            out=t_[:, 0:4, :],
                            in_=xT_d[:, 0:4, qc * 512:(qc + 1) * 512])
                        nc.sync.dma_start(
                            out=t_[:, 4:NC, :],
                            in_=xT_d[:, 4:NC, qc * 512:(qc + 1) * 512])
                    for c in range(NC):
                        xq[(c, qc)] = t_[:, c, :]
                return [one]

            def alloc_weights():
                wsb = {}
                for wtag in ("wq", "wk", "wv"):
                    wsb[wtag] = p_w.tile([128, NC, FPC], BF16, tag=wtag,
                                         name=wtag)
                wp_t = p_wp.tile([128, NF, C], BF16, tag="wp", name="wp")
                return wsb, wp_t

            def dma_w_halves(dst, src_ap):
                nc.sync.dma_start(out=dst[:, 0:4, :], in_=src_ap[:, 0:4, :])
                nc.sync.dma_start(out=dst[:, 4:NC, :], in_=src_ap[:, 4:NC, :])

            def q_group(qc, f, isq):
                wsb = wq_sb if isq else wk_sb
                ps = ps_q.tile([128, 512], F32, tag="q", name=f"qk{qc}_{f}")
                for c in range(NC):
                    nc.tensor.matmul(
                        ps[:], wsb[c][:, f * 128:(f + 1) * 128],
                        xq[(c, qc)][:], start=(c == 0), stop=(c == NC - 1))
                if isq:
                    dst = p_qtq.tile([128, 512], BF16, tag="qt",
                                     name=f"qtq{f}_{qc}")
                    qtq[(f, qc)] = dst
                    nc.vector.tensor_copy(out=dst[:], in_=ps[:])
                else:
                    nc.vector.tensor_copy(
                        out=kt_[f][:, qc * 512:(qc + 1) * 512], in_=ps[:])

            def v_group(qc, t):
                ps = ps_q.tile([128, FPC], F32, tag="q", name=f"v{t}")
                for c in range(NC):
                    nc.tensor.matmul(
                        ps[:], xq[(c, qc)][:, (t % 4) * 128:(t % 4 + 1) * 128],
                        wv_sb[c][:], start=(c == 0), stop=(c == NC - 1))
                nc.vector.tensor_copy(
                    out=v65[t][:, :, 0:64],
                    in_=ps[:].rearrange("p (h d) -> p h d", h=HPC))
                nc.gpsimd.tensor_copy(
                    out=v65[t][:, :, 64:65],
                    in_=onesv[:].rearrange("p (h o) -> p h o", h=HPC))

            def qkv_steps(qc):
                gs = []
                for f in range(NF):
                    gs.append(lambda f=f: q_group(qc, f, True))
                for f in range(NF):
                    gs.append(lambda f=f: q_group(qc, f, False))
                for t in range(4 * qc, 4 * qc + 4):
                    gs.append(lambda t=t: v_group(qc, t))
                return gs

            def qkv_hp_steps(qc, f):
                return [lambda: q_group(qc, f, True),
                        lambda: q_group(qc, f, False)]

            def v_steps(qc):
                return [lambda t=t: v_group(qc, t)
                        for t in range(4 * qc, 4 * qc + 4)]

            def attn_steps(qc):
                nk = 4 * qc + 4
                steps = []
                y_ps = {}

                def av_step(hp, kt):
                    hA, hB = 2 * hp, 2 * hp + 1
                    if kt == 0:
                        y_ps[hA] = ps_y.tile([65, 512], F32, tag="y",
                                             name=f"y{hA}_{qc}")
                        y_ps[hB] = ps_y.tile([65, 512], F32, tag="y",
                                             name=f"y{hB}_{qc}")
                    d = kt - 4 * qc
                    qoff = 128 * d if d > 0 else 0
                    sp = ps_s.tile([128, 1024], F32, tag="s")
                    for s in (0, 1):
                        nc.tensor.matmul(
                            sp[:, 512 * s + qoff:512 * s + 512],
                            kt_[hp][64 * s:64 * s + 64,
                                    kt * 128:(kt + 1) * 128],
                            qtq[(hp, qc)][64 * s:64 * s + 64, qoff:512],
                            start=True, stop=True, tile_position=(64 * s, 0))
                    pt = p_pt.tile([128, 1024], BF16, tag="pt")
                    nc.scalar.activation(
                        out=pt[:].rearrange("p (s w) -> p s w",
                                            s=2)[:, :, qoff:512],
                        in_=sp[:].rearrange("p (s w) -> p s w",
                                            s=2)[:, :, qoff:512],
                        func=EXP, scale=float(SCALE))
                    if d >= 0:
                        nc.vector.tensor_mul(
                            pt[:].rearrange("p (s w) -> p s w", s=2)
                            [:, :, qoff:qoff + 128],
                            pt[:].rearrange("p (s w) -> p s w", s=2)
                            [:, :, qoff:qoff + 128],
                            trimask2[:])
                    for s, h in ((0, hA), (1, hB)):
                        nc.tensor.matmul(
                            y_ps[h][:, qoff:512], v65[kt][:, h, :],
                            pt[:, 512 * s + qoff:512 * s + 512],
                            start=(kt == 0), stop=(kt == nk - 1),
                            skip_group_check=True)

                def divide(hp):
                    hA, hB = 2 * hp, 2 * hp + 1
                    recA = p_rec.tile([65, 512], F32R, tag="rec")
                    recB = p_rec.tile([65, 512], F32R, tag="rec")
                    with nc.allow_low_precision("f32r softmax reciprocal"):
                        nc.vector.reciprocal(out=recA[64:65, :],
                                             in_=y_ps[hA][64:65, :])
                        nc.vector.reciprocal(out=recB[64:65, :],
                                             in_=y_ps[hB][64:65, :])
                    ysA = p_ys.tile([64, 512], F32, tag="ys")
                    nc.vector.tensor_copy(out=ysA[:], in_=y_ps[hA][0:64, :])
                    ysB = p_ys.tile([64, 512], F32, tag="ys")
                    nc.vector.tensor_copy(out=ysB[:], in_=y_ps[hB][0:64, :])
                    bcA = ps_q.tile([64, 512], F32, tag="q", name=f"bc{hA}_{qc}")
                    nc.tensor.matmul(
                        bcA[:], ones_t[64:65, :], recA[64:65, :],
                        start=True, stop=True, tile_position=(64, 0))
                    bcB = ps_q.tile([64, 512], F32, tag="q", name=f"bc{hB}_{qc}")
                    nc.tensor.matmul(
                        bcB[:], ones_t[64:65, :], recB[64:65, :],
                        start=True, stop=True, tile_position=(64, 0))
                    yt = p_ytq.tile([128, 512], BF16, tag="yt",
                                    name=f"ytq{hp}_{qc}")
                    ytq[(hp, qc)] = yt
                    nc.vector.tensor_mul(yt[0:64, :], ysA[:], bcA[:])
                    ybt = p_ybt.tile([64, 512], BF16, tag="ybt")
                    nc.vector.tensor_mul(ybt[:], ysB[:], bcB[:])
                    nc.sync.dma_start(out=yt[64:128, :], in_=ybt[:])

                chains = []
                for hp in range(NF):
                    ch = []
                    for kt in range(nk):
                        ch.append(lambda hp=hp, kt=kt: av_step(hp, kt))
                    ch.append(lambda hp=hp: divide(hp))
                    chains.append(ch)
                return chains

            def proj_steps(qc, pool=None):
                pool_, ptag = pool or ps_q, "q"
                steps = []
                boxes = {}

                def half(t, nn):
                    if nn == 0:
                        boxes[t] = p_osb.tile([128, C], BF16, tag="osb",
                                              name=f"osb{t}")
                    osb = boxes[t]
                    tloc = (t - 4 * qc) * 128
                    pj = pool_.tile([128, 512], F32, tag=ptag,
                                    name=f"pj{t}_{nn}")
                    for cf in range(NF):
                        nc.tensor.matmul(
                            pj[:], ytq[(cf, qc)][:, tloc:tloc + 128],
                            wp_sb[cf][:, nn * 512:(nn + 1) * 512],
                            start=(cf == 0), stop=(cf == NF - 1))
                    nc.vector.tensor_copy(
                        out=osb[:, nn * 512:(nn + 1) * 512], in_=pj[:])
                    nc.sync.dma_start(
                        out=out_d[t * 128:(t + 1) * 128,
                                  nn * 512:(nn + 1) * 512],
                        in_=osb[:, nn * 512:(nn + 1) * 512])

                for t in range(4 * qc, 4 * qc + 4):
                    for nn in range(2):
                        steps.append(lambda t=t, nn=nn: half(t, nn))
                return steps

            # prologue: x block 0 + wq, then QKV q-groups while wk/wv/wp
            # stream in behind them
            wsb, wp_t = alloc_weights()
            wq_sb = [wsb["wq"][:, c, :] for c in range(NC)]
            wk_sb = [wsb["wk"][:, c, :] for c in range(NC)]
            wv_sb = [wsb["wv"][:, c, :] for c in range(NC)]
            wp_sb = [wp_t[:, cf, :] for cf in range(NF)]
            t0_ = p_xq.tile([128, NC, 512], BF16, tag="xq", name="xq0")
            for c in range(NC):
                xq[(c, 0)] = t0_[:, c, :]
            for c4 in range(0, NC, 2):
                if serdep["on"] and c4 == 0:
                    nc.sync.dma_start(
                        out=t0_[:, 0:1, :],
                        in_=out_d[(NT - 1) * 128:NT * 128, 0:512]
                        .rearrange("p (o f) -> p o f", o=1))
                    nc.sync.dma_start(out=t0_[:, 1:2, :],
                                      in_=xT_d[:, 1:2, 0:512])
                else:
                    nc.sync.dma_start(out=t0_[:, c4:c4 + 2, :],
                                      in_=xT_d[:, c4:c4 + 2, 0:512])
                nc.sync.dma_start(out=wsb["wq"][:, c4:c4 + 2, :],
                                  in_=wq_d[:, c4:c4 + 2, :])
            qs = qkv_steps(0)
            qs[0]()
            dma_w_halves(wsb["wk"], wk_d)
            qs[1]()
            dma_w_halves(wsb["wv"], wv_d)
            nc.sync.dma_start(out=wp_t[:], in_=wp_d[:])
            load_consts()
            for st in qs[2:]:
                st()

            def run_window(main, inject):
                ii, L, M = 0, len(main), len(inject)
                for i, st in enumerate(main):
                    st()
                    while ii * L < (i + 1) * M:
                        inject[ii]()
                        ii += 1

            # pipelined main loop. Windows 0/1: attention(qc) chains with
            # xq/QKV(qc+1) (+proj(qc-1)) injected. Blocks 2 and 3 are merged
            # at hp-chain granularity — block 3's attention steps are
            # ACT(exp)-paced, so alternating them with block 2's chains and
            # the projection streams keeps PE saturated through the tail.
            a0 = attn_steps(0)
            a1 = attn_steps(1)
            run_window(sum(a0, []), dma_xq_steps(1) + qkv_steps(1))
            run_window(sum(a1, []), dma_xq_steps(2) + qkv_steps(2))
            a2 = attn_steps(2)
            a3 = attn_steps(3)
            # each block-3 piece lands just before its consumer chain:
            # a3[hp] reads qtq(hp,3)/kt_[hp] block 3, and v65[12..15]
            p0 = proj_steps(0)
            p1 = proj_steps(1, pool=ps_q)
            run_window(a2[0],
                       dma_xq_steps(3) + qkv_hp_steps(3, 0) + v_steps(3))
            run_window(a3[0] + a2[1], qkv_hp_steps(3, 1) + p0[:4])
            run_window(a3[1] + a2[2], qkv_hp_steps(3, 2) + p0[4:])
            run_window(a3[2] + a2[3], qkv_hp_steps(3, 3) + p1)
            run_window(a3[3], proj_steps(2, pool=ps_q))
            for st in proj_steps(NQ - 1, pool=ps_q):
                st()

        for _rep in range(repeat):
            serdep["on"] = serialize_reps and _rep > 0
            emit_once()
    nc.finalize()
    return nc


def _make_masks():
    kk = np.arange(128)[:, None]
    jj = np.arange(128)[None, :]
    return (jj >= kk).astype(ml_dtypes.bfloat16)


def _bf16(a):
    return np.asarray(a, dtype=np.float32).astype(ml_dtypes.bfloat16)


def _pack(a, ncols):
    """[n*128, ncols] -> [128, n, ncols] (partition-major chunk packing)."""
    a = np.asarray(a, dtype=np.float32)
    n = a.shape[0] // 128
    return np.ascontiguousarray(
        a.reshape(n, 128, ncols).transpose(1, 0, 2)).astype(ml_dtypes.bfloat16)


def make_in_maps(x, W_qkv, W_proj):
    """Host-side sharding of full inputs into per-core input maps."""
    x = np.asarray(x, dtype=np.float32)
    W_qkv = np.asarray(W_qkv, dtype=np.float32)
    W_proj = np.asarray(W_proj, dtype=np.float32)
    masks = _make_masks()
    in_maps = []
    for core in range(NCORES):
        b, g = core // GROUPS, core % GROUPS
        in_maps.append({
            "xT": _pack(x[b].T, T),
            "wq": _pack(W_qkv[:, g * FPC:(g + 1) * FPC], FPC),
            "wk": _pack(W_qkv[:, C + g * FPC:C + (g + 1) * FPC], FPC),
            "wv": _pack(W_qkv[:, 2 * C + g * FPC:2 * C + (g + 1) * FPC], FPC),
            "wp": _pack(W_proj[g * FPC:(g + 1) * FPC, :], C),
            "trimask": masks,
            "ones64": np.ones((1, 64), np.float32),
            "onesv": np.ones((128, HPC), ml_dtypes.bfloat16),
        })
    return in_maps


_CACHE = {}


def _get_nc():
    if "nc" not in _CACHE:
        _CACHE["nc"] = build_nc()
    return _CACHE["nc"]


def run_cores(in_maps):
    res = run_bass_kernel_spmd(_get_nc(), in_maps, list(range(NCORES)))
    return res.results


def assemble(results):
    out = np.empty((B, T, C), dtype=np.float32)
    for b in range(B):
        out[b] = np.asarray(results[GROUPS * b]["out"], dtype=np.float32)
        for g in range(1, GROUPS):
            out[b] += np.asarray(results[GROUPS * b + g]["out"],
                                 dtype=np.float32)
    return out


def kernel(x, W_qkv, W_proj):
    return assemble(run_cores(make_in_maps(x, W_qkv, W_proj)))


# revision 37
# speedup vs baseline: 1.0854x; 1.0854x over previous
"""Causal self-attention TRN2 kernel (bf16, software-pipelined).

Full module: x[4,2048,1024] @ W_qkv[1024,3072] -> heads(16, d=64) causal attn
-> @ W_proj[1024,1024].

Sharding: 8 cores = 4 batches x 2 head-groups (8 heads each), tensor-parallel
over heads. Each core computes q/k/v for its 8 heads, causal attention, and a
partial projection (row-sharded W_proj). The two partials per batch are summed
on the host (no on-device collectives).

Design (v1 measured 540us/pass on HW; this version ~370us; TimelineSim 279us
with PE busy 236us ~= the causal-attention FLOP roofline at 78.6 TF/s bf16):
  - All matmul operands bf16: PE rate equals f32r, no low-pstate penalty,
    half the SBUF/DMA footprint. End-to-end bf16 numpy model: 0.41% max err;
    measured 0.38% on HW vs the 2% gate (output is returned as bf16 and
    upcast on the host, +0.2%).
  - Diagonal skip: for a diagonal k-tile at block offset d, the first 128*d
    q-columns are fully masked -> S matmul, exp and AV all run on the partial
    width only. No zero-fill DMAs (v1 fetched zeros from HBM on the exp->AV
    critical path).
  - Attention processed per (head-pair, k-tile) step: two 64-contraction S
    matmuls write one [128,1024] PSUM tile (head A | head B), ONE exp covers
    both heads, one strided DVE multiply applies the causal mask to both
    diagonal blocks, two AV matmuls accumulate per-head [65,512] y tiles
    (V|ones trick: row 64 = softmax denominator). Fewer, larger ops measured
    faster on HW than per-head [128,512] steps - cross-engine semaphore
    latency dominates over the simulator's estimates.
  - PSUM: 8 banks = scores 2x2 + y-accum 2 + qkv/proj/bcast 2. Dedicated
    pools per phase (v1 shared one pool, serializing QKV behind projection).
  - Software-pipelined emission: per-block windows interleave attention(qc)
    with xq-prefetch/QKV(qc+1) and projection(qc-2..) steps; blocks 2 and 3
    are merged at hp-chain granularity and all projection work is pushed into
    the late (exp-paced) windows so PE stays saturated. Weight/x loads are
    one or two big host-packed DMAs each ([128, chunk, cols] layout).
  - PSUM->SBUF staging: QKV copies on ACT (idle between exps in the early
    windows), projection copies and the softmax divide chain on DVE, mask
    multiplies on DVE, v-ones fill on GPSIMD. GPSIMD cannot access PSUM.

build_nc(repeat=R) emits the whole computation R times (rep r+1's first xq
chunk is sourced from rep r's final output rows, serializing reps) for
wall-clock-differenced device timing.
"""

import numpy as np
import ml_dtypes
from contextlib import ExitStack

import concourse.bass as bass
import concourse.tile as tile
from concourse import mybir, bacc
from concourse.bass_utils import run_bass_kernel_spmd

F32 = mybir.dt.float32
F32R = mybir.dt.float32r
BF16 = mybir.dt.bfloat16
EXP = mybir.ActivationFunctionType.Exp

B, T, C, H, D = 4, 2048, 1024, 16, 64
NCORES = 8
GROUPS = 2            # head groups (tensor-parallel dimension)
HPC = H // GROUPS     # heads per core = 8
FPC = HPC * D         # features per core = 512
SCALE = 1.0 / np.sqrt(D)


def build_nc(repeat=1, serialize_reps=False):
    NC = C // 128     # contraction chunks over C = 8
    NT = T // 128     # token tiles (also k-tiles) = 16
    NQ = T // 512     # query chunks = 4
    NF = FPC // 128   # feature tiles = head pairs = 4
    nc = bacc.Bacc("TRN2", debug=False)
    xT_d = nc.dram_tensor("xT", [128, NC, T], BF16, kind="ExternalInput").ap()
    wq_d = nc.dram_tensor("wq", [128, NC, FPC], BF16, kind="ExternalInput").ap()
    wk_d = nc.dram_tensor("wk", [128, NC, FPC], BF16, kind="ExternalInput").ap()
    wv_d = nc.dram_tensor("wv", [128, NC, FPC], BF16, kind="ExternalInput").ap()
    wp_d = nc.dram_tensor("wp", [128, NF, C], BF16, kind="ExternalInput").ap()
    mk_d = nc.dram_tensor("trimask", [128, 128], BF16, kind="ExternalInput").ap()
    on_d = nc.dram_tensor("ones64", [1, 64], F32R, kind="ExternalInput").ap()
    ov_d = nc.dram_tensor("onesv", [128, HPC], BF16, kind="ExternalInput").ap()
    out_d = nc.dram_tensor("out", [T, C], BF16, kind="ExternalOutput").ap()

    with tile.TileContext(nc) as tc, ExitStack() as ctx:
        p_const = ctx.enter_context(tc.tile_pool(name="p_const", bufs=1))
        p_kt = ctx.enter_context(tc.tile_pool(name="p_kt", bufs=NF))
        p_v65 = ctx.enter_context(tc.tile_pool(name="p_v65", bufs=NT))
        p_w = ctx.enter_context(tc.tile_pool(name="p_w", bufs=3))
        p_wp = ctx.enter_context(tc.tile_pool(name="p_wp", bufs=NF))
        p_xq = ctx.enter_context(tc.tile_pool(name="p_xq", bufs=2))
        p_qtq = ctx.enter_context(tc.tile_pool(name="p_qtq", bufs=2 * NF))
        p_ytq = ctx.enter_context(tc.tile_pool(name="p_ytq", bufs=4 * NF))
        p_pt = ctx.enter_context(tc.tile_pool(name="p_pt", bufs=4))
        p_ys = ctx.enter_context(tc.tile_pool(name="p_ys", bufs=4))
        p_rec = ctx.enter_context(tc.tile_pool(name="p_rec", bufs=2))
        p_ybt = ctx.enter_context(tc.tile_pool(name="p_ybt", bufs=2))
        p_osb = ctx.enter_context(tc.tile_pool(name="p_osb", bufs=2))
        # PSUM: 8 banks = scores 2 + y-accum 2 + qkv 2 + proj/bcast 2
        ps_s = ctx.enter_context(tc.tile_pool(name="ps_s", bufs=2, space="PSUM"))
        ps_y = ctx.enter_context(tc.tile_pool(name="ps_y", bufs=2, space="PSUM"))
        ps_q = ctx.enter_context(tc.tile_pool(name="ps_q", bufs=2, space="PSUM"))

        # constants (loaded once, outside the repeat loop, after the first
        # x/wq quarters — they are not needed until attention starts)
        ones_t = p_const.tile([65, 64], F32R, tag="ones")
        trimask2 = p_const.tile([128, 2, 128], BF16, tag="trimask2")
        onesv = p_const.tile([128, HPC], BF16, tag="onesv")
        consts = {"loaded": False}

        def load_consts():
            if consts["loaded"]:
                return
            consts["loaded"] = True
            nc.sync.dma_start(out=ones_t[64:65, :], in_=on_d[:])
            nc.sync.dma_start(out=trimask2[:, 0, :], in_=mk_d[:])
            nc.sync.dma_start(out=trimask2[:, 1, :], in_=mk_d[:])
            nc.sync.dma_start(out=onesv[:], in_=ov_d[:])

        kt_ = [p_kt.tile([128, T], BF16, tag="kt", name=f"kt{i}")
               for i in range(NF)]
        v65 = [p_v65.tile([128, HPC, 65], BF16, tag="v65", name=f"v65_{i}")
               for i in range(NT)]

        serdep = {"on": False}

        def emit_once():
            qtq = {}   # (f, qc) -> [128, 512] bf16 query quarter (transposed)
            ytq = {}   # (hp, qc) -> [128, 512] bf16 attention-out quarter
            xq = {}    # (c, qc) -> [128, 512] bf16 x chunk (transposed)
            wq_sb = wk_sb = wv_sb = wp_sb = None

            def dma_xq_steps(qc):
                def one():
                    t_ = p_xq.tile([128, NC, 512], BF16, tag="xq",
                                   name=f"xq{qc}")
                    if serdep["on"] and qc == 0:
                        # timing mode: source chunk 0 from the previous
                        # repeat's output rows — every q/k/v contraction
                        # chain starts at chunk 0, so reps serialize
                        nc.sync.dma_start(
                            out=t_[:, 0:1, :],
                            in_=out_d[(NT - 1) * 128:NT * 128, 0:512]
                            .rearrange("p (o f) -> p o f", o=1))
                        nc.sync.dma_start(
                            out=t_[:, 1:NC, :],
                            in_=xT_d[:, 1:NC, qc * 512:(qc + 1) * 512])
                    elif qc == 0:
                        for c4 in range(0, NC, 2):
                            nc.sync.dma_start(
                                out=t_[:, c4:c4 + 2, :],
                                in_=xT_d[:, c4:c4 + 2,
                                         qc * 512:(qc + 1) * 512])
                    else:
                        nc.sync.dma_start(
                            out=t_[:, 0:4, :],
                            in_=xT_d[:, 0:4, qc * 512:(qc + 1) * 512])
                        nc.sync.dma_start(
                            out=t_[:, 4:NC, :],
                            in_=xT_d[:, 4:NC, qc * 512:(qc + 1) * 512])
                    for c in range(NC):
                        xq[(c, qc)] = t_[:, c, :]
                return [one]

            def alloc_weights():
                wsb = {}
                for wtag in ("wq", "wk", "wv"):
                    wsb[wtag] = p_w.tile([128, NC, FPC], BF16, tag=wtag,
                                         name=wtag)
                wp_t = p_wp.tile([128, NF, C], BF16, tag="wp", name="wp")
                return wsb, wp_t

            def dma_w_halves(dst, src_ap):
                nc.sync.dma_start(out=dst[:, 0:4, :], in_=src_ap[:, 0:4, :])
                nc.sync.dma_start(out=dst[:, 4:NC, :], in_=src_ap[:, 4:NC, :])

            def q_group(qc, f, isq):
                wsb = wq_sb if isq else wk_sb
                ps = ps_q.tile([128, 512], F32, tag="q", name=f"qk{qc}_{f}")
                for c in range(NC):
                    nc.tensor.matmul(
                        ps[:], wsb[c][:, f * 128:(f + 1) * 128],
                        xq[(c, qc)][:], start=(c == 0), stop=(c == NC - 1))
                if isq:
                    dst = p_qtq.tile([128, 512], BF16, tag="qt",
                                     name=f"qtq{f}_{qc}")
                    qtq[(f, qc)] = dst
                    nc.scalar.copy(out=dst[:], in_=ps[:])
                else:
                    nc.scalar.copy(
                        out=kt_[f][:, qc * 512:(qc + 1) * 512], in_=ps[:])

            def v_group(qc, t):
                ps = ps_q.tile([128, FPC], F32, tag="q", name=f"v{t}")
                for c in range(NC):
                    nc.tensor.matmul(
                        ps[:], xq[(c, qc)][:, (t % 4) * 128:(t % 4 + 1) * 128],
                        wv_sb[c][:], start=(c == 0), stop=(c == NC - 1))
                nc.scalar.copy(
                    out=v65[t][:, :, 0:64],
                    in_=ps[:].rearrange("p (h d) -> p h d", h=HPC))
                nc.gpsimd.tensor_copy(
                    out=v65[t][:, :, 64:65],
                    in_=onesv[:].rearrange("p (h o) -> p h o", h=HPC))

            def qkv_steps(qc):
                gs = []
                for f in range(NF):
                    gs.append(lambda f=f: q_group(qc, f, True))
                for f in range(NF):
                    gs.append(lambda f=f: q_group(qc, f, False))
                for t in range(4 * qc, 4 * qc + 4):
                    gs.append(lambda t=t: v_group(qc, t))
                return gs

            def qkv_hp_steps(qc, f):
                return [lambda: q_group(qc, f, True),
                        lambda: q_group(qc, f, False)]

            def v_steps(qc):
                return [lambda t=t: v_group(qc, t)
                        for t in range(4 * qc, 4 * qc + 4)]

            def attn_steps(qc):
                nk = 4 * qc + 4
                steps = []
                y_ps = {}

                def av_step(hp, kt):
                    hA, hB = 2 * hp, 2 * hp + 1
                    if kt == 0:
                        y_ps[hA] = ps_y.tile([65, 512], F32, tag="y",
                                             name=f"y{hA}_{qc}")
                        y_ps[hB] = ps_y.tile([65, 512], F32, tag="y",
                                             name=f"y{hB}_{qc}")
                    d = kt - 4 * qc
                    qoff = 128 * d if d > 0 else 0
                    sp = ps_s.tile([128, 1024], F32, tag="s")
                    for s in (0, 1):
                        nc.tensor.matmul(
                            sp[:, 512 * s + qoff:512 * s + 512],
                            kt_[hp][64 * s:64 * s + 64,
                                    kt * 128:(kt + 1) * 128],
                            qtq[(hp, qc)][64 * s:64 * s + 64, qoff:512],
                            start=True, stop=True, tile_position=(64 * s, 0))
                    pt = p_pt.tile([128, 1024], BF16, tag="pt")
                    nc.scalar.activation(
                        out=pt[:].rearrange("p (s w) -> p s w",
                                            s=2)[:, :, qoff:512],
                        in_=sp[:].rearrange("p (s w) -> p s w",
                                            s=2)[:, :, qoff:512],
                        func=EXP, scale=float(SCALE))
                    if d >= 0:
                        nc.vector.tensor_mul(
                            pt[:].rearrange("p (s w) -> p s w", s=2)
                            [:, :, qoff:qoff + 128],
                            pt[:].rearrange("p (s w) -> p s w", s=2)
                            [:, :, qoff:qoff + 128],
                            trimask2[:])
                    for s, h in ((0, hA), (1, hB)):
                        nc.tensor.matmul(
                            y_ps[h][:, qoff:512], v65[kt][:, h, :],
                            pt[:, 512 * s + qoff:512 * s + 512],
                            start=(kt == 0), stop=(kt == nk - 1),
                            skip_group_check=True)

                def divide(hp):
                    hA, hB = 2 * hp, 2 * hp + 1
                    recA = p_rec.tile([65, 512], F32R, tag="rec")
                    recB = p_rec.tile([65, 512], F32R, tag="rec")
                    with nc.allow_low_precision("f32r softmax reciprocal"):
                        nc.vector.reciprocal(out=recA[64:65, :],
                                             in_=y_ps[hA][64:65, :])
                        nc.vector.reciprocal(out=recB[64:65, :],
                                             in_=y_ps[hB][64:65, :])
                    ysA = p_ys.tile([64, 512], F32, tag="ys")
                    nc.vector.tensor_copy(out=ysA[:], in_=y_ps[hA][0:64, :])
                    ysB = p_ys.tile([64, 512], F32, tag="ys")
                    nc.vector.tensor_copy(out=ysB[:], in_=y_ps[hB][0:64, :])
                    bcA = ps_q.tile([64, 512], F32, tag="q", name=f"bc{hA}_{qc}")
                    nc.tensor.matmul(
                        bcA[:], ones_t[64:65, :], recA[64:65, :],
                        start=True, stop=True, tile_position=(64, 0))
                    bcB = ps_q.tile([64, 512], F32, tag="q", name=f"bc{hB}_{qc}")
                    nc.tensor.matmul(
                        bcB[:], ones_t[64:65, :], recB[64:65, :],
                        start=True, stop=True, tile_position=(64, 0))
                    yt = p_ytq.tile([128, 512], BF16, tag="yt",
                                    name=f"ytq{hp}_{qc}")
                    ytq[(hp, qc)] = yt
                    nc.vector.tensor_mul(yt[0:64, :], ysA[:], bcA[:])
                    ybt = p_ybt.tile([64, 512], BF16, tag="ybt")
                    nc.vector.tensor_mul(ybt[:], ysB[:], bcB[:])
                    nc.sync.dma_start(out=yt[64:128, :], in_=ybt[:])

                chains = []
                for hp in range(NF):
                    ch = []
                    for kt in range(nk):
                        ch.append(lambda hp=hp, kt=kt: av_step(hp, kt))
                    ch.append(lambda hp=hp: divide(hp))
                    chains.append(ch)
                return chains

            def proj_steps(qc, pool=None):
                pool_, ptag = pool or ps_q, "q"
                steps = []
                boxes = {}

                def half(t, nn):
                    if nn == 0:
                        boxes[t] = p_osb.tile([128, C], BF16, tag="osb",
                                              name=f"osb{t}")
                    osb = boxes[t]
                    tloc = (t - 4 * qc) * 128
                    pj = pool_.tile([128, 512], F32, tag=ptag,
                                    name=f"pj{t}_{nn}")
                    for cf in range(NF):
                        nc.tensor.matmul(
                            pj[:], ytq[(cf, qc)][:, tloc:tloc + 128],
                            wp_sb[cf][:, nn * 512:(nn + 1) * 512],
                            start=(cf == 0), stop=(cf == NF - 1))
                    nc.vector.tensor_copy(
                        out=osb[:, nn * 512:(nn + 1) * 512], in_=pj[:])
                    nc.sync.dma_start(
                        out=out_d[t * 128:(t + 1) * 128,
                                  nn * 512:(nn + 1) * 512],
                        in_=osb[:, nn * 512:(nn + 1) * 512])

                for t in range(4 * qc, 4 * qc + 4):
                    for nn in range(2):
                        steps.append(lambda t=t, nn=nn: half(t, nn))
                return steps

            # prologue: x block 0 + wq, then QKV q-groups while wk/wv/wp
            # stream in behind them
            wsb, wp_t = alloc_weights()
            wq_sb = [wsb["wq"][:, c, :] for c in range(NC)]
            wk_sb = [wsb["wk"][:, c, :] for c in range(NC)]
            wv_sb = [wsb["wv"][:, c, :] for c in range(NC)]
            wp_sb = [wp_t[:, cf, :] for cf in range(NF)]
            t0_ = p_xq.tile([128, NC, 512], BF16, tag="xq", name="xq0")
            for c in range(NC):
                xq[(c, 0)] = t0_[:, c, :]
            for c4 in range(0, NC, 2):
                if serdep["on"] and c4 == 0:
                    nc.sync.dma_start(
                        out=t0_[:, 0:1, :],
                        in_=out_d[(NT - 1) * 128:NT * 128, 0:512]
                        .rearrange("p (o f) -> p o f", o=1))
                    nc.sync.dma_start(out=t0_[:, 1:2, :],
                                      in_=xT_d[:, 1:2, 0:512])
                else:
                    nc.sync.dma_start(out=t0_[:, c4:c4 + 2, :],
                                      in_=xT_d[:, c4:c4 + 2, 0:512])
                nc.sync.dma_start(out=wsb["wq"][:, c4:c4 + 2, :],
                                  in_=wq_d[:, c4:c4 + 2, :])
            qs = qkv_steps(0)
            qs[0]()
            dma_w_halves(wsb["wk"], wk_d)
            qs[1]()
            dma_w_halves(wsb["wv"], wv_d)
            nc.sync.dma_start(out=wp_t[:], in_=wp_d[:])
            load_consts()
            for st in qs[2:]:
                st()

            def run_window(main, inject):
                ii, L, M = 0, len(main), len(inject)
                for i, st in enumerate(main):
                    st()
                    while ii * L < (i + 1) * M:
                        inject[ii]()
                        ii += 1

            # pipelined main loop. Windows 0/1: attention(qc) chains with
            # xq/QKV(qc+1) (+proj(qc-1)) injected. Blocks 2 and 3 are merged
            # at hp-chain granularity — block 3's attention steps are
            # ACT(exp)-paced, so alternating them with block 2's chains and
            # the projection streams keeps PE saturated through the tail.
            a0 = attn_steps(0)
            a1 = attn_steps(1)
            run_window(sum(a0, []), dma_xq_steps(1) + qkv_steps(1))
            run_window(sum(a1, []), dma_xq_steps(2) + qkv_steps(2))
            a2 = attn_steps(2)
            a3 = attn_steps(3)
            # each block-3 piece lands just before its consumer chain:
            # a3[hp] reads qtq(hp,3)/kt_[hp] block 3, and v65[12..15]
            p0 = proj_steps(0)
            p1 = proj_steps(1, pool=ps_q)
            run_window(a2[0],
                       dma_xq_steps(3) + qkv_hp_steps(3, 0) + v_steps(3))
            run_window(a3[0] + a2[1], qkv_hp_steps(3, 1) + p0[:4])
            run_window(a3[1] + a2[2], qkv_hp_steps(3, 2) + p0[4:])
            run_window(a3[2] + a2[3], qkv_hp_steps(3, 3) + p1)
            run_window(a3[3], proj_steps(2, pool=ps_q))
            for st in proj_steps(NQ - 1, pool=ps_q):
                st()

        for _rep in range(repeat):
            serdep["on"] = serialize_reps and _rep > 0
            emit_once()
    nc.finalize()
    return nc


def _make_masks():
    kk = np.arange(128)[:, None]
    jj = np.arange(128)[None, :]
    return (jj >= kk).astype(ml_dtypes.bfloat16)


def _bf16(a):
    return np.asarray(a, dtype=np.float32).astype(ml_dtypes.bfloat16)


def _pack(a, ncols):
    """[n*128, ncols] -> [128, n, ncols] (partition-major chunk packing)."""
    a = np.asarray(a, dtype=np.float32)
    n = a.shape[0] // 128
    return np.ascontiguousarray(
        a.reshape(n, 128, ncols).transpose(1, 0, 2)).astype(ml_dtypes.bfloat16)


def make_in_maps(x, W_qkv, W_proj):
    """Host-side sharding of full inputs into per-core input maps."""
    x = np.asarray(x, dtype=np.float32)
    W_qkv = np.asarray(W_qkv, dtype=np.float32)
    W_proj = np.asarray(W_proj, dtype=np.float32)
    masks = _make_masks()
    in_maps = []
    for core in range(NCORES):
        b, g = core // GROUPS, core % GROUPS
        in_maps.append({
            "xT": _pack(x[b].T, T),
            "wq": _pack(W_qkv[:, g * FPC:(g + 1) * FPC], FPC),
            "wk": _pack(W_qkv[:, C + g * FPC:C + (g + 1) * FPC], FPC),
            "wv": _pack(W_qkv[:, 2 * C + g * FPC:2 * C + (g + 1) * FPC], FPC),
            "wp": _pack(W_proj[g * FPC:(g + 1) * FPC, :], C),
            "trimask": masks,
            "ones64": np.ones((1, 64), np.float32),
            "onesv": np.ones((128, HPC), ml_dtypes.bfloat16),
        })
    return in_maps


_CACHE = {}


def _get_nc():
    if "nc" not in _CACHE:
        _CACHE["nc"] = build_nc()
    return _CACHE["nc"]


def run_cores(in_maps):
    res = run_bass_kernel_spmd(_get_nc(), in_maps, list(range(NCORES)))
    return res.results


def assemble(results):
    out = np.empty((B, T, C), dtype=np.float32)
    for b in range(B):
        out[b] = np.asarray(results[GROUPS * b]["out"], dtype=np.float32)
        for g in range(1, GROUPS):
            out[b] += np.asarray(results[GROUPS * b + g]["out"],
                                 dtype=np.float32)
    return out


def kernel(x, W_qkv, W_proj):
    return assemble(run_cores(make_in_maps(x, W_qkv, W_proj)))
